# revision 12
# baseline (speedup 1.0000x reference)
"""Trainium2 Bass kernel for nn_DemoPredictor (GRU + negative-sampling loss).

Data-parallel over batch B=512 across 8 NeuronCores (64 rows each).
Per core:
  - embedding rows gathered from the replicated item_emb table via
    indirect DMA, 128 rows (= 2 timesteps x 64 batch) per transfer
  - gathered rows PE-transposed to E-major; input projection
    gi = embed @ w_ih.T + biases per 2-step pair into PSUM (fp32r
    matmuls, biases folded in via K=1 ones-outer-product matmuls),
    copied to SBUF
  - sequential GRU over T=200 steps: gh = h @ w_hh.T as fp32r matmuls
    streaming w_hh (stationary hT); gates split engine-wise:
    sigmoids on ACT (tanh via 2*sigmoid(2x)-1 to stay in one ACT
    table), PSUM-coupled elementwise on DVE, SBUF-only combines on
    DVE/GpSimd; h' PE-transposed back to K-major for the next step
  - h' streamed to a DRAM history each step; the last-valid-step row
    per batch is fetched back by one indirect DMA, then the output
    head + log-sigmoid loss reduce on device (softplus = ln(1+exp)).
Per-core [sum_contrib, sum_valid] combined on host.
"""
import numpy as np

import concourse.bass as bass
import concourse.mybir as mybir
from concourse import bacc, tile
from concourse.masks import make_identity
from concourse.bass_utils import run_bass_kernel_spmd

F32 = mybir.dt.float32
F32R = mybir.dt.float32r
I32 = mybir.dt.int32
AF = mybir.ActivationFunctionType
OP = mybir.AluOpType

B, T = 512, 200
E, H = 256, 512
VOCAB = 100000
L, NN = 20, 4
NCORES = 8
BL = B // NCORES          # 64
PAIRS = T // 2            # 100
H3 = 3 * H                # 1536
EC = E // 128             # 2
HC = H // 128             # 4
HH = H // 2               # 256 (half for n-path pipelining)


def build_nc():
    nc = bacc.Bacc(
        "TRN2", target_bir_lowering=False, debug=False, enable_asserts=False
    )
    emb = nc.dram_tensor("emb", [VOCAB, E], F32, kind="ExternalInput")
    xidx = nc.dram_tensor("xidx", [128, PAIRS], I32, kind="ExternalInput")
    wihT = nc.dram_tensor("wihT", [E, H3], F32, kind="ExternalInput")
    whhT = nc.dram_tensor("whhT", [H, H3], F32, kind="ExternalInput")
    biases = nc.dram_tensor("biases", [1, 2048], F32, kind="ExternalInput")
    woutT = nc.dram_tensor("woutT", [H, L], F32, kind="ExternalInput")
    lastidx = nc.dram_tensor("lastidx", [BL, 1], I32, kind="ExternalInput")
    yv = nc.dram_tensor("yv", [BL, L], F32, kind="ExternalInput")
    ob = nc.dram_tensor("ob", [BL, L], F32, kind="ExternalInput")
    negs = nc.dram_tensor("negs", [BL, NN * L], F32, kind="ExternalInput")
    out = nc.dram_tensor("out", [2, 1], F32, kind="ExternalOutput")
    h_hist = nc.dram_tensor("h_hist", [T * BL, H], F32, kind="Internal")

    with tile.TileContext(nc) as tc:
        with (
            tc.tile_pool(name="const", bufs=1) as cp,
            tc.tile_pool(name="raw", bufs=3) as rawp,
            tc.tile_pool(name="embt", bufs=2) as embtp,
            tc.tile_pool(name="gis", bufs=2) as gisp,
            tc.tile_pool(name="state", bufs=2) as stp,
            tc.tile_pool(name="gates", bufs=2) as gp,
            tc.tile_pool(name="fin", bufs=1) as fp_,
            tc.tile_pool(name="ps_gi", bufs=1, space="PSUM") as ps_gi,
            tc.tile_pool(name="ps_rec", bufs=1, space="PSUM") as ps_rec,
            tc.tile_pool(name="ps_rb", bufs=1, space="PSUM") as ps_rb,
            tc.tile_pool(name="ps_gb", bufs=1, space="PSUM") as ps_gb,
            tc.tile_pool(name="ps_ht", bufs=1, space="PSUM") as ps_ht,
            tc.tile_pool(name="ps_ht2", bufs=1, space="PSUM") as ps_ht2,
        ):
            # ---- constants / one-time loads ----
            ident = cp.tile([128, 128], F32)
            make_identity(nc, ident[:])
            idx_sb = cp.tile([128, PAIRS], I32)
            nc.sync.dma_start(out=idx_sb[:], in_=xidx[:])
            lidx_sb = cp.tile([BL, 1], I32)
            nc.sync.dma_start(out=lidx_sb[:], in_=lastidx[:])
            wih_sb = cp.tile([128, EC * H3], F32R)
            nc.gpsimd.dma_start(
                out=wih_sb[:].rearrange("p (c n) -> p c n", c=EC),
                in_=wihT[:].rearrange("(c p) n -> p c n", p=128),
            )
            whh_sb = cp.tile([128, HC * H3], F32R)
            nc.gpsimd.dma_start(
                out=whh_sb[:].rearrange("p (c n) -> p c n", c=HC),
                in_=whhT[:].rearrange("(c p) n -> p c n", p=128),
            )
            bias_sb = cp.tile([1, 2048], F32R)
            nc.gpsimd.dma_start(out=bias_sb[:], in_=biases[:])
            ones_f = cp.tile([1, 128], F32)
            nc.vector.memset(ones_f[:], 1.0)
            ones1 = cp.tile([1, 128], F32R)
            nc.vector.tensor_copy(ones1[:], ones_f[:])
            y_sb = cp.tile([BL, L], F32)
            nc.sync.dma_start(out=y_sb[:], in_=yv[:])
            ob_sb = cp.tile([BL, L], F32)
            nc.sync.dma_start(out=ob_sb[:], in_=ob[:])
            negs_sb = cp.tile([BL, NN * L], F32)
            nc.sync.dma_start(out=negs_sb[:], in_=negs[:])
            wout_sb = cp.tile([128, HC * L], F32)
            nc.sync.dma_start(
                out=wout_sb[:].rearrange("p (c n) -> p c n", c=HC),
                in_=woutT[:].rearrange("(c p) n -> p c n", p=128),
            )
            h0 = cp.tile([BL, H], F32)
            nc.vector.memset(h0[:], 0.0)

            h_cur = h0
            hT_cur = None

            def gi_pair(p):
                """Gather + transpose + input projection for pair p.
                Returns dict gate -> SBUF tile [128, 512] incl. biases
                (rows 0:64 = step 2p, rows 64:128 = step 2p+1)."""
                raw = rawp.tile([128, E], F32, tag="raw")
                nc.gpsimd.indirect_dma_start(
                    out=raw[:],
                    out_offset=None,
                    in_=emb[:],
                    in_offset=bass.IndirectOffsetOnAxis(
                        ap=idx_sb[:, p:p + 1], axis=0
                    ),
                )
                embt = embtp.tile([128, EC * 128], F32R, tag="embt")
                for c in range(EC):
                    tp = ps_gi.tile([128, 128], F32, space="PSUM", tag="gi")
                    nc.tensor.transpose(
                        tp[:], raw[:, c * 128:(c + 1) * 128], ident[:]
                    )
                    nc.scalar.copy(embt[:, c * 128:(c + 1) * 128], tp[:])
                # only the n-gate gi goes through SBUF; r/z accumulate
                # their gi directly in the per-step PSUM banks
                bank = ps_gi.tile([128, 512], F32, space="PSUM", tag="gi")
                nc.tensor.matmul(
                    bank[:], ones1[:], bias_sb[:, 1024:1536],
                    start=True, stop=False, skip_group_check=True,
                )
                for c in range(EC):
                    nc.tensor.matmul(
                        bank[:],
                        embt[:, c * 128:(c + 1) * 128],
                        wih_sb[:, c * H3 + 1024:c * H3 + 1536],
                        start=False, stop=(c == EC - 1),
                        skip_group_check=True,
                    )
                gin_sb = gisp.tile([128, 512], F32, tag="gin")
                nc.scalar.copy(gin_sb[:], bank[:])
                return {"embt": embt, "gin": gin_sb}

            gi_cur = gi_pair(0)
            gi_next = None

            for t in range(T):
                p, half = divmod(t, 2)
                lo = half * BL
                embt = gi_cur["embt"]
                gin = gi_cur["gin"][lo:lo + BL, :]

                # r/z banks: bias + gi + gh all accumulated in PSUM
                rb = ps_rb.tile([BL, 512], F32, space="PSUM", tag="rb")
                zb = ps_rec.tile([BL, 512], F32, space="PSUM", tag="zb")
                gb0 = ps_gb.tile([BL, HH], F32, space="PSUM", tag="gb0")
                gb1 = ps_gb.tile([BL, HH], F32, space="PSUM", tag="gb1")
                gbs = [gb0, gb1]
                for bank, bofs in ((rb, 0), (zb, 512)):
                    nc.tensor.matmul(
                        bank[:], ones1[:, :BL], bias_sb[:, bofs:bofs + 512],
                        start=True, stop=False, skip_group_check=True,
                    )
                    for c in range(EC):
                        nc.tensor.matmul(
                            bank[:],
                            embt[:, c * 128 + lo:c * 128 + lo + BL],
                            wih_sb[:, c * H3 + bofs:c * H3 + bofs + 512],
                            start=False, stop=(t == 0 and c == EC - 1),
                            skip_group_check=True,
                        )
                for hh in range(2):
                    nc.tensor.matmul(
                        gbs[hh][:], ones1[:, :BL],
                        bias_sb[:, 1536 + hh * HH:1536 + (hh + 1) * HH],
                        start=True, stop=(t == 0), skip_group_check=True,
                    )
                def hT_chunk(c):
                    return hT_cur[c // 2][:, (c % 2) * BL:(c % 2 + 1) * BL]

                if t > 0:
                    for c in range(HC):
                        nc.tensor.matmul(
                            rb[:],
                            hT_chunk(c),
                            whh_sb[:, c * H3:c * H3 + 512],
                            start=False, stop=(c == HC - 1),
                            skip_group_check=True,
                        )
                    for hh in range(2):
                        for c in range(HC):
                            nc.tensor.matmul(
                                gbs[hh][:],
                                hT_chunk(c),
                                whh_sb[:, c * H3 + 1024 + hh * HH:
                                       c * H3 + 1024 + (hh + 1) * HH],
                                start=False,
                                stop=(c == HC - 1),
                                skip_group_check=True,
                            )
                    for c in range(HC):
                        nc.tensor.matmul(
                            zb[:],
                            hT_chunk(c),
                            whh_sb[:, c * H3 + 512:c * H3 + 1024],
                            start=False, stop=(c == HC - 1),
                            skip_group_check=True,
                        )

                # gates: sigmoids straight from PSUM
                r_t = gp.tile([BL, H], F32, tag="r")
                s_t = gp.tile([BL, H], F32, tag="s")  # s = 1 - z
                nc.scalar.activation(r_t[:], rb[:], AF.Sigmoid)
                nc.scalar.activation(s_t[:], zb[:], AF.Sigmoid, scale=-1.0)
                # q = s + (s-1)*h  (so h' = 2*s*n2 - q); off the n-chain
                w_t = gp.tile([BL, H], F32, tag="w")
                q_t = gp.tile([BL, H], F32, tag="q")
                for hh in range(2):
                    sl = slice(hh * HH, (hh + 1) * HH)
                    nc.vector.scalar_tensor_tensor(
                        out=w_t[:, sl], in0=s_t[:, sl], scalar=1.0,
                        in1=h_cur[:, sl], op0=OP.subtract, op1=OP.mult,
                    )
                    nc.gpsimd.tensor_add(q_t[:, sl], s_t[:, sl], w_t[:, sl])

                v_t = gp.tile([BL, H], F32, tag="v")
                n2_t = gp.tile([BL, H], F32, tag="n2")
                h_new = stp.tile([BL, H], F32, tag="h")
                hT_new = [None, None]
                for hh in range(2):
                    sl = slice(hh * HH, (hh + 1) * HH)
                    # u = r*ghn in-place in PSUM, v = u + gin
                    gbh = gbs[hh]
                    nc.vector.tensor_mul(gbh[:], r_t[:, sl], gbh[:])
                    nc.vector.tensor_add(v_t[:, sl], gbh[:], gin[:, sl])
                    # n = 2*sigmoid(2v) - 1 (= tanh(v)); fold the 2x and
                    # -1 into c2/h': h' = s*n + (1-s)h = 2*s*n2 - q
                    nc.scalar.activation(
                        n2_t[:, sl], v_t[:, sl], AF.Sigmoid, scale=2.0
                    )
                    c2 = gp.tile([BL, HH], F32, tag=f"c2{hh}")
                    nc.vector.scalar_tensor_tensor(
                        out=c2[:], in0=n2_t[:, sl], scalar=2.0,
                        in1=s_t[:, sl], op0=OP.mult, op1=OP.mult,
                    )
                    nc.vector.tensor_sub(h_new[:, sl], c2[:], q_t[:, sl])
                    if t < T - 1:
                        pool = ps_ht if hh == 0 else ps_ht2
                        htp = pool.tile([128, 2 * BL], F32, space="PSUM",
                                        tag=f"ht{hh}")
                        for ci, c in enumerate((2 * hh, 2 * hh + 1)):
                            nc.tensor.transpose(
                                htp[:, ci * BL:(ci + 1) * BL],
                                h_new[:, c * 128:(c + 1) * 128],
                                ident[:BL, :BL],
                            )
                        hT_h = stp.tile([128, 2 * BL], F32R, tag=f"hT{hh}")
                        nc.vector.tensor_copy(hT_h[:], htp[:])
                        hT_new[hh] = hT_h
                nc.sync.dma_start(
                    out=h_hist[t * BL:(t + 1) * BL, :], in_=h_new[:]
                )
                if t < T - 1:
                    hT_cur = hT_new
                h_cur = h_new

                if half == 0 and p + 1 < PAIRS:
                    gi_next = gi_pair(p + 1)
                elif half == 1:
                    gi_cur = gi_next

            # ---- epilogue: fetch user_rep, head + loss ----
            ur = fp_.tile([BL, H], F32)
            nc.gpsimd.indirect_dma_start(
                out=ur[:],
                out_offset=None,
                in_=h_hist[:],
                in_offset=bass.IndirectOffsetOnAxis(ap=lidx_sb[:, :1], axis=0),
            )
            urT_ps = ps_ht.tile([128, HC * BL], F32, space="PSUM", tag="ht")
            for c in range(HC):
                nc.tensor.transpose(
                    urT_ps[:, c * BL:(c + 1) * BL],
                    ur[:, c * 128:(c + 1) * 128],
                    ident[:BL, :BL],
                )
            urT = fp_.tile([128, HC * BL], F32)
            nc.vector.tensor_copy(urT[:], urT_ps[:])
            wu_ps = ps_rb.tile([BL, L], F32, space="PSUM", tag="rb")
            for c in range(HC):
                nc.tensor.matmul(
                    wu_ps[:],
                    urT[:, c * BL:(c + 1) * BL],
                    wout_sb[:, c * L:(c + 1) * L],
                    start=(c == 0), stop=(c == HC - 1),
                )
            wc = fp_.tile([BL, L], F32)
            nc.vector.tensor_mul(wc[:], wu_ps[:], ob_sb[:])
            # softplus(x) = ln(1 + exp(x)); args are O(1), no overflow
            py = fp_.tile([BL, L], F32)
            nc.vector.tensor_mul(py[:], wc[:], y_sb[:])
            e1 = fp_.tile([BL, L], F32)
            nc.scalar.activation(e1[:], py[:], AF.Exp, scale=-1.0)
            e1p = fp_.tile([BL, L], F32)
            nc.vector.tensor_scalar_add(e1p[:], e1[:], 1.0)
            lg1 = fp_.tile([BL, L], F32)
            pos_sum = fp_.tile([BL, 1], F32)
            nc.scalar.activation(lg1[:], e1p[:], AF.Ln, accum_out=pos_sum[:])
            m_t = fp_.tile([BL, NN * L], F32)
            for k in range(NN):
                nc.vector.tensor_mul(
                    m_t[:, k * L:(k + 1) * L],
                    negs_sb[:, k * L:(k + 1) * L],
                    wc[:],
                )
            e2 = fp_.tile([BL, NN * L], F32)
            nc.scalar.activation(e2[:], m_t[:], AF.Exp)
            e2p = fp_.tile([BL, NN * L], F32)
            nc.vector.tensor_scalar_add(e2p[:], e2[:], 1.0)
            lg2 = fp_.tile([BL, NN * L], F32)
            neg_sum = fp_.tile([BL, 1], F32)
            nc.scalar.activation(lg2[:], e2p[:], AF.Ln, accum_out=neg_sum[:])
            vs = fp_.tile([BL, 1], F32)
            nc.vector.tensor_reduce(
                vs[:], wc[:], axis=mybir.AxisListType.X, op=OP.add
            )
            va = fp_.tile([BL, 1], F32)
            nc.scalar.activation(va[:], vs[:], AF.Abs)
            valid = fp_.tile([BL, 1], F32)
            nc.scalar.activation(valid[:], va[:], AF.Sign)
            tot = fp_.tile([BL, 1], F32)
            nc.vector.tensor_add(tot[:], pos_sum[:], neg_sum[:])
            red_in = fp_.tile([BL, 2], F32)
            nc.vector.tensor_mul(red_in[:, 0:1], tot[:], valid[:])
            nc.vector.tensor_copy(red_in[:, 1:2], valid[:])
            onesB = fp_.tile([BL, 1], F32)
            nc.vector.memset(onesB[:], 1.0)
            red_ps = ps_rec.tile([2, 1], F32, space="PSUM", tag="zb")
            nc.tensor.matmul(
                red_ps[:], red_in[:], onesB[:], start=True, stop=True
            )
            red_sb = fp_.tile([2, 1], F32)
            nc.vector.tensor_copy(red_sb[:], red_ps[:])
            nc.sync.dma_start(out=out[:], in_=red_sb[:])
    nc.finalize()
    return nc


_NC_CACHE = None


def _get_nc():
    global _NC_CACHE
    if _NC_CACHE is None:
        _NC_CACHE = build_nc()
    return _NC_CACHE


def make_in_maps(x, x_mask, y, ob, neg_samples, item_emb,
                 w_ih, w_hh, b_ih, b_hh, w_out):
    """Host-side sharding / layout prep. Returns list of 8 input dicts."""
    item_emb = np.ascontiguousarray(np.asarray(item_emb, dtype=np.float32))
    wihT = np.ascontiguousarray(np.asarray(w_ih, np.float32).T)   # [E, 3H]
    whhT = np.ascontiguousarray(np.asarray(w_hh, np.float32).T)   # [H, 3H]
    b_ih = np.asarray(b_ih, np.float32)
    b_hh = np.asarray(b_hh, np.float32)
    brz = b_ih[:2 * H] + b_hh[:2 * H]
    biases = np.concatenate(
        [brz, b_ih[2 * H:], b_hh[2 * H:]]
    ).reshape(1, 2048).astype(np.float32)
    woutT = np.ascontiguousarray(np.asarray(w_out, np.float32).T)  # [H, L]
    x = np.asarray(x)
    x_len = np.asarray(x_mask).astype(np.int64).sum(axis=1)        # [B]
    last = np.clip(x_len - 1, 0, T - 1).astype(np.int64)           # [B]
    y = np.asarray(y, np.float32)
    ob = np.asarray(ob, np.float32)
    negs = np.asarray(neg_samples, np.float32).reshape(B, NN * L)

    in_maps = []
    for c in range(NCORES):
        rs = slice(c * BL, (c + 1) * BL)
        xl = x[rs].astype(np.int32)                     # [BL, T]
        xt = xl.T.reshape(PAIRS, 2, BL)                 # [p, half, b]
        xidx = np.ascontiguousarray(
            xt.transpose(1, 2, 0).reshape(128, PAIRS)
        )
        lastidx = (
            last[rs] * BL + np.arange(BL)
        ).astype(np.int32).reshape(BL, 1)
        in_maps.append({
            "emb": item_emb,
            "xidx": xidx,
            "wihT": wihT,
            "whhT": whhT,
            "biases": biases,
            "woutT": woutT,
            "lastidx": lastidx,
            "yv": np.ascontiguousarray(y[rs]),
            "ob": np.ascontiguousarray(ob[rs]),
            "negs": np.ascontiguousarray(negs[rs]),
        })
    return in_maps


def combine_outputs(results):
    num = sum(float(r["out"][0, 0]) for r in results)
    nva = sum(float(r["out"][1, 0]) for r in results)
    return np.float32(num / max(nva, 1.0))


def kernel(**inputs) -> np.ndarray:
    nc = _get_nc()
    in_maps = make_in_maps(**inputs)
    res = run_bass_kernel_spmd(nc, in_maps, core_ids=list(range(NCORES)))
    return np.asarray(combine_outputs(res.results))
